# revision 1
# baseline (speedup 1.0000x reference)
"""BinaryDense forward kernel for Trainium2 (8 NeuronCores, data-parallel).

Computes y = x @ w_bin + bias where w_bin is the stochastic binarization
({-1,+1}) of a 128x128 weight matrix (fixed bernoulli key 42, matching the
jax reference bit-exactly; computed on host since it is tiny).

Sharding: x [2097152, 128] f32 is split along M into 8 shards of
[262144, 128], one per NeuronCore; w_bin and bias are replicated.

Per-core kernel (all fp32, exact):
  - SBUF partition p holds a contiguous block of 2048 rows of the shard, so
    every HBM<->SBUF DMA moves 16 KiB contiguous per partition per chunk.
  - Each [128,128] row-tile is transposed on the TensorEngine (transpose
    mode, exact pass-through) into PSUM, evicted to SBUF by the ScalarE,
    then used as the stationary operand of a fp32 matmul against the
    replicated w_bin; y lands in PSUM with M on partitions and is evicted
    (+bias, pre-tiled on host to [128,512]) by the VectorE, then DMA'd out.
"""

import numpy as np

P = 128  # partitions
K = 128  # contraction dim
N = 128  # output features
M_TOTAL = 2097152
NCORES = 8
M_LOCAL = M_TOTAL // NCORES  # 262144
ROWS_PER_PART = M_LOCAL // P  # 2048 rows of x per SBUF partition
T = 32  # row-tiles per chunk (each tile = 128 rows spread across partitions)
NCHUNK = ROWS_PER_PART // T  # 64 chunks per core
GROUP = 4  # tiles per PSUM bank
NG = T // GROUP  # groups per chunk
CH = T * K  # free elements per chunk per partition

_cache = {}


def _binarize_weight(weight: np.ndarray, is_training) -> np.ndarray:
    """Reproduce the reference's stochastic binarization bit-exactly."""
    training = bool(np.asarray(is_training).item())
    if not training:
        return np.where(weight > 0, 1.0, -1.0).astype(np.float32)
    import jax
    import jax.numpy as jnp

    with jax.default_device(jax.devices("cpu")[0]):
        w = jnp.asarray(weight, dtype=jnp.float32)
        prob_pos = jnp.clip((w + 1.0) / 2.0, 0.0, 1.0)
        bern = jax.random.bernoulli(jax.random.key(42), prob_pos, w.shape)
        w_bin = jnp.where(bern, 1.0, -1.0).astype(jnp.float32)
        return np.asarray(w_bin)


def _build():
    if "nc" in _cache:
        return _cache["nc"]

    import concourse.bass as bass
    import concourse.tile as tile
    from concourse import bacc, mybir
    from concourse.masks import make_identity

    fdt = mybir.dt.float32
    nc = bacc.Bacc(
        "TRN2", target_bir_lowering=False, debug=False, num_devices=NCORES
    )
    x_h = nc.declare_dram_parameter("x", [M_LOCAL, K], fdt, isOutput=False)
    w_h = nc.declare_dram_parameter("w", [K, N], fdt, isOutput=False)
    b_h = nc.declare_dram_parameter("bias4", [P, GROUP * N], fdt, isOutput=False)
    y_h = nc.declare_dram_parameter("y", [M_LOCAL, N], fdt, isOutput=True)

    with tile.TileContext(nc) as tc:
        from contextlib import ExitStack

        with ExitStack() as ctx:
            xv = x_h.ap().rearrange("(p r) k -> p (r k)", p=P)
            yv = y_h.ap().rearrange("(p r) k -> p (r k)", p=P)

            const = ctx.enter_context(tc.tile_pool(name="const", bufs=1))
            identity = const.tile([P, P], fdt)
            make_identity(nc, identity)
            w_t = const.tile([K, N], fdt)
            nc.sync.dma_start(w_t, w_h.ap())
            bias_t = const.tile([P, GROUP * N], fdt)
            nc.sync.dma_start(bias_t, b_h.ap())

            xin_pool = ctx.enter_context(tc.tile_pool(name="xin", bufs=2))
            yout_pool = ctx.enter_context(tc.tile_pool(name="yout", bufs=2))
            xt_pool = ctx.enter_context(tc.tile_pool(name="xt", bufs=12))
            psT_pool = ctx.enter_context(
                tc.tile_pool(name="psT", bufs=3, space="PSUM")
            )
            psM_pool = ctx.enter_context(
                tc.tile_pool(name="psM", bufs=3, space="PSUM")
            )

            for c in range(NCHUNK):
                xin = xin_pool.tile([P, CH], fdt)
                nc.sync.dma_start(xin, xv[:, c * CH : (c + 1) * CH])
                yout = yout_pool.tile([P, CH], fdt)
                xts = []
                for g in range(NG):
                    psT = psT_pool.tile([P, GROUP * P], fdt)
                    for i in range(GROUP):
                        r = g * GROUP + i
                        nc.tensor.transpose(
                            psT[:, i * P : (i + 1) * P],
                            xin[:, r * K : (r + 1) * K],
                            identity,
                        )
                    xt = xt_pool.tile([P, GROUP * P], fdt)
                    nc.scalar.copy(xt, psT)
                    xts.append(xt)
                for g in range(NG):
                    psM = psM_pool.tile([P, GROUP * N], fdt)
                    for i in range(GROUP):
                        nc.tensor.matmul(
                            psM[:, i * N : (i + 1) * N],
                            xts[g][:, i * P : (i + 1) * P],
                            w_t,
                            start=True,
                            stop=True,
                        )
                    nc.vector.tensor_add(
                        yout[:, g * GROUP * N : (g + 1) * GROUP * N], psM, bias_t
                    )
                nc.scalar.dma_start(yv[:, c * CH : (c + 1) * CH], yout)

    nc.compile()
    _cache["nc"] = nc
    return nc


def kernel(x, weight, bias, is_training):
    x = np.ascontiguousarray(np.asarray(x, dtype=np.float32))
    weight = np.asarray(weight, dtype=np.float32)
    bias = np.asarray(bias, dtype=np.float32)
    assert x.shape == (M_TOTAL, K), x.shape

    w_bin = _binarize_weight(weight, is_training)
    bias4 = np.ascontiguousarray(np.tile(bias[None, :], (P, GROUP)).astype(np.float32))

    nc = _build()
    from concourse.bass_utils import run_bass_kernel_spmd

    in_maps = [
        {
            "x": x[i * M_LOCAL : (i + 1) * M_LOCAL],
            "w": w_bin,
            "bias4": bias4,
        }
        for i in range(NCORES)
    ]
    res = run_bass_kernel_spmd(nc, in_maps, list(range(NCORES)))
    y = np.concatenate([res.results[i]["y"] for i in range(NCORES)], axis=0)
    return y.astype(np.float32)


# revision 4
# speedup vs baseline: 80.5329x; 80.5329x over previous
"""BinaryDense forward kernel for Trainium2 (8 NeuronCores, data-parallel).

Computes y = x @ w_bin + bias where w_bin is the stochastic binarization
({-1,+1}) of a 128x128 weight matrix (fixed bernoulli key 42, matching the
jax reference bit-exactly; computed on host since it is tiny).

Sharding: x [2097152, 128] f32 is split along M into 8 shards of
[262144, 128], one per NeuronCore; w_bin and bias are replicated.

Per-core kernel (all fp32, exact):
  - SBUF partition p holds a contiguous block of 2048 rows of the shard, so
    every HBM<->SBUF DMA moves 16 KiB contiguous per partition per chunk.
  - Each [128,128] row-tile is transposed on the TensorEngine (transpose
    mode, exact pass-through) into PSUM, evicted to SBUF by the ScalarE,
    then used as the stationary operand of a fp32 matmul against the
    replicated w_bin; y lands in PSUM with M on partitions and is evicted
    (+bias, pre-tiled on host to [128,512]) by the VectorE, then DMA'd out.
"""

import numpy as np

P = 128  # partitions
K = 128  # contraction dim
N = 128  # output features
M_TOTAL = 2097152
NCORES = 8
M_LOCAL = M_TOTAL // NCORES  # 262144
ROWS_PER_PART = M_LOCAL // P  # 2048 rows of x per SBUF partition
T = 32  # row-tiles per chunk (each tile = 128 rows spread across partitions)
NCHUNK = ROWS_PER_PART // T  # 64 chunks per core
GROUP = 4  # tiles per PSUM bank
NG = T // GROUP  # groups per chunk
CH = T * K  # free elements per chunk per partition

_cache = {}


def _binarize_weight(weight: np.ndarray, is_training) -> np.ndarray:
    """Reproduce the reference's stochastic binarization bit-exactly."""
    training = bool(np.asarray(is_training).item())
    if not training:
        return np.where(weight > 0, 1.0, -1.0).astype(np.float32)
    import jax
    import jax.numpy as jnp

    with jax.default_device(jax.devices("cpu")[0]):
        w = jnp.asarray(weight, dtype=jnp.float32)
        prob_pos = jnp.clip((w + 1.0) / 2.0, 0.0, 1.0)
        bern = jax.random.bernoulli(jax.random.key(42), prob_pos, w.shape)
        w_bin = jnp.where(bern, 1.0, -1.0).astype(jnp.float32)
        return np.asarray(w_bin)


def _build(reps: int = 1):
    key = ("nc", reps)
    if key in _cache:
        return _cache[key]

    import concourse.tile as tile
    from concourse import bacc, mybir
    from concourse.masks import make_identity
    from contextlib import ExitStack

    fdt = mybir.dt.float32
    nc = bacc.Bacc("TRN2", target_bir_lowering=False, debug=False, num_devices=NCORES)
    x_h = nc.declare_dram_parameter("x", [M_LOCAL, K], fdt, isOutput=False)
    w_h = nc.declare_dram_parameter("w", [K, N], fdt, isOutput=False)
    b_h = nc.declare_dram_parameter("bias4", [P, GROUP * N], fdt, isOutput=False)
    y_h = nc.declare_dram_parameter("y", [M_LOCAL, N], fdt, isOutput=True)

    with tile.TileContext(nc) as tc, ExitStack() as ctx:
        xv = x_h.ap().rearrange("(p r) k -> p (r k)", p=P)
        yv = y_h.ap().rearrange("(p r) k -> p (r k)", p=P)

        const = ctx.enter_context(tc.tile_pool(name="const", bufs=1))
        identity = const.tile([P, P], fdt)
        make_identity(nc, identity)
        w_t = const.tile([K, N], fdt)
        nc.sync.dma_start(w_t, w_h.ap())
        bias_t = const.tile([P, GROUP * N], fdt)
        nc.sync.dma_start(bias_t, b_h.ap())

        xin_pool = ctx.enter_context(tc.tile_pool(name="xin", bufs=2))
        yout_pool = ctx.enter_context(tc.tile_pool(name="yout", bufs=2))
        xt_pool = ctx.enter_context(tc.tile_pool(name="xt", bufs=12))
        psT_pool = ctx.enter_context(tc.tile_pool(name="psT", bufs=3, space="PSUM"))
        psM_pool = ctx.enter_context(tc.tile_pool(name="psM", bufs=3, space="PSUM"))

        for _rep in range(reps):
            for c in range(NCHUNK):
                xin = xin_pool.tile([P, CH], fdt)
                nc.sync.dma_start(xin, xv[:, c * CH : (c + 1) * CH])
                yout = yout_pool.tile([P, CH], fdt)
                xts = []
                for g in range(NG):
                    psT = psT_pool.tile([P, GROUP * P], fdt)
                    for i in range(GROUP):
                        r = g * GROUP + i
                        nc.tensor.transpose(
                            psT[:, i * P : (i + 1) * P],
                            xin[:, r * K : (r + 1) * K],
                            identity,
                        )
                    xt = xt_pool.tile([P, GROUP * P], fdt)
                    nc.scalar.copy(xt, psT)
                    xts.append(xt)
                for g in range(NG):
                    psM = psM_pool.tile([P, GROUP * N], fdt)
                    for i in range(GROUP):
                        nc.tensor.matmul(
                            psM[:, i * N : (i + 1) * N],
                            xts[g][:, i * P : (i + 1) * P],
                            w_t,
                            start=True,
                            stop=True,
                        )
                    nc.vector.tensor_add(
                        yout[:, g * GROUP * N : (g + 1) * GROUP * N], psM, bias_t
                    )
                nc.scalar.dma_start(yv[:, c * CH : (c + 1) * CH], yout)

    nc.compile()
    _cache[key] = nc
    return nc


def kernel(x, weight, bias, is_training):
    x = np.ascontiguousarray(np.asarray(x, dtype=np.float32))
    weight = np.asarray(weight, dtype=np.float32)
    bias = np.asarray(bias, dtype=np.float32)
    assert x.shape == (M_TOTAL, K), x.shape

    w_bin = _binarize_weight(weight, is_training)
    bias4 = np.ascontiguousarray(np.tile(bias[None, :], (P, GROUP)).astype(np.float32))

    nc = _build()
    from concourse.bass_utils import run_bass_kernel_spmd

    in_maps = [
        {
            "x": x[i * M_LOCAL : (i + 1) * M_LOCAL],
            "w": w_bin,
            "bias4": bias4,
        }
        for i in range(NCORES)
    ]
    res = run_bass_kernel_spmd(nc, in_maps, list(range(NCORES)))
    y = np.concatenate([res.results[i]["y"] for i in range(NCORES)], axis=0)
    return y.astype(np.float32)
